# revision 1
# baseline (speedup 1.0000x reference)
"""GPT-2 attention (B=16, S=1024, E=768, H=12, D=64) on 8 TRN2 NeuronCores.

Sharding: data-parallel over batch — each core processes B_LOC=2 batch
elements with fully replicated weights. No collectives.

Per-core pipeline (per batch element):
  A. x [1024,768] -> x^T in SBUF via PE transposes (48 [128,128] tiles)
  B. v = x @ W_attn[:,1536:2304] + b  (seq-major, heads interleaved with a
     fused ones-column per head for softmax denominators)
  C. per head-pair t (q/k feature tile):
     C1. q^T, k^T = (x @ W)^T via lhsT=W chunks, rhs=x^T  (feature-major)
     C2. per head: scores^T bands (lhsT=k^T slice, rhs=q^T) -> Exp(s/8) from
         PSUM -> causal tri-mask on diagonal block -> attn@v with fused
         colsum (M=65) accumulating over bands -> reciprocal -> K=1
         outer-product broadcast -> normalize into attn_out^T
  D. out = attn_out @ W_proj + b_proj (bias via K=1 outer product into PSUM)

All matmuls run in float32r (full-rate PE, ~1.6e-4 rel err measured on HW).
"""

import sys

sys.path.insert(0, "/opt/trn_rl_repo")

from contextlib import ExitStack

import numpy as np

import concourse.bass as bass
import concourse.mybir as mybir
import concourse.tile as tile
from concourse.masks import make_identity

F32 = mybir.dt.float32
F32R = mybir.dt.float32r
BF16 = mybir.dt.bfloat16
AF = mybir.ActivationFunctionType

B, S, E = 16, 1024, 768
H, D = 12, 64
NCORES = 8
B_LOC = B // NCORES          # 2 batch elements per core
KC = E // 128                # 6 contraction chunks
ST = S // 128                # 8 seq tiles
PAIRS = H // 2               # 6 head pairs (2 heads per 128-row feature tile)


def emit(tc, outs, ins):
    nc = tc.nc
    x, wa, ba, wp, bp = (ins["hidden_states"], ins["W_attn"], ins["b_attn"],
                         ins["W_proj"], ins["b_proj"])
    out = outs["out"]
    # weights/biases are consumed as f32r matmul operands; the real build()
    # declares them f32r in DRAM, the sim harness allocates fp32 — reinterpret
    wa = wa if wa.dtype == F32R else wa.bitcast(F32R)
    ba = ba if ba.dtype == F32R else ba.bitcast(F32R)
    wp = wp if wp.dtype == F32R else wp.bitcast(F32R)
    bp = bp if bp.dtype == F32R else bp.bitcast(F32R)

    ctx = ExitStack()
    with ctx:
        wpool = ctx.enter_context(tc.tile_pool(name="wpool", bufs=1))
        work = ctx.enter_context(tc.tile_pool(name="work", bufs=1))
        ps = ctx.enter_context(tc.tile_pool(name="ps", bufs=2, space="PSUM"))

        # ---------- persistent weights (f32r via in-place rounding copy)
        wa_r = []
        for k in range(KC):
            w = wpool.tile([128, 3 * E], F32R, tag=f"wa{k}", name=f"wa{k}")
            nc.sync.dma_start(w, wa[k * 128:(k + 1) * 128, :])
            wa_r.append(w)
        wp_r = []
        for k in range(KC):
            w = wpool.tile([128, E], F32R, tag=f"wp{k}", name=f"wp{k}")
            nc.sync.dma_start(w, wp[k * 128:(k + 1) * 128, :])
            wp_r.append(w)

        # q/k bias, feature-major [128, 12]: (p, m) = b_attn[m*128 + p]
        ba_qk = wpool.tile([128, 2 * KC], F32)
        nc.sync.dma_start(ba_qk.bitcast(F32R),
                          ba[0:2 * E].rearrange("(m p) -> p m", p=128))
        # v bias and proj bias as rows (outer-product rhs), f32r
        ba_v = wpool.tile([1, E], F32R)
        nc.sync.dma_start(ba_v, ba[2 * E:3 * E].unsqueeze(0))
        bp_r = wpool.tile([1, E], F32R)
        nc.sync.dma_start(bp_r, bp.unsqueeze(0))

        identity = wpool.tile([128, 128], F32)
        make_identity(nc, identity)

        ones_col32 = wpool.tile([128, 1], F32)
        nc.vector.memset(ones_col32, 1.0)
        ones_row32 = wpool.tile([1, 128], F32)
        nc.vector.memset(ones_row32, 1.0)
        ones_row = wpool.tile([1, 128], F32R)
        nc.vector.tensor_copy(ones_row, ones_row32)

        # pre-broadcast v/proj biases to [128, E] so bias-add fuses into the
        # PSUM->SBUF copy on DVE (replaces per-tile K=1 outer products on PE)
        biasv_bc = wpool.tile([128, E], F32)
        biasp_bc = wpool.tile([128, E], F32)
        for bc_dst, brow in ((biasv_bc, ba_v), (biasp_bc, bp_r)):
            for n0, nw in ((0, 512), (512, 256)):
                bps = ps.tile([128, 512], F32, tag="tr", name=f"bbc{n0}_{brow.name}")
                nc.tensor.matmul(bps[:, 0:nw], ones_row, brow[0:1, n0:n0 + nw],
                                 start=True, stop=True)
                nc.scalar.activation(bc_dst[:, n0:n0 + nw], bps[:, 0:nw], AF.Copy)

        # causal tri-mask [128,128]: mask[r,c] = 1 if c >= r else 0
        trimask32 = wpool.tile([128, 128], F32)
        nc.gpsimd.memset(trimask32, 1.0)
        nc.gpsimd.affine_select(
            out=trimask32, in_=trimask32, compare_op=mybir.AluOpType.is_ge,
            fill=0.0, base=0, pattern=[[1, 128]], channel_multiplier=-1,
        )
        trimask = wpool.tile([128, 128], BF16)
        nc.vector.tensor_copy(trimask, trimask32)

        for b in range(B_LOC):
            # ---------- A: x^T via PE transposes
            xT = []
            for k in range(KC):
                t_ = work.tile([128, S], F32R, tag=f"xt{k}", name=f"xT{k}_{b}")
                xT.append(t_)
            for k in range(KC):
                for st in range(ST):
                    xin = work.tile([128, 128], F32, tag="xin", bufs=3,
                                    name=f"xin{b}_{k}_{st}")
                    nc.sync.dma_start(
                        xin, x[b, st * 128:(st + 1) * 128, k * 128:(k + 1) * 128])
                    tr_ps = ps.tile([128, 128], F32, tag="tr", name=f"tr{b}_{k}_{st}")
                    nc.tensor.transpose(tr_ps, xin, identity)
                    nc.scalar.activation(
                        xT[k][:, st * 128:(st + 1) * 128], tr_ps, AF.Copy)

            # ---------- B: v (seq-major, 12 heads x [64 cols + ones col])
            v_r = []
            for st in range(ST):
                vt = work.tile([128, H, D + 1], BF16, tag=f"v{st}", name=f"v{st}_{b}")
                v_r.append(vt)
                nc.vector.tensor_copy(
                    vt[:, :, D:D + 1], ones_col32.broadcast_to((128, H, 1)))
                for n0, nw in ((0, 512), (512, 256)):
                    acc = ps.tile([128, 512], F32, tag="acc", name=f"vacc{b}_{st}_{n0}")
                    for k in range(KC):
                        nc.tensor.matmul(
                            acc[:, 0:nw],
                            xT[k][:, st * 128:(st + 1) * 128],
                            wa_r[k][:, 2 * E + n0:2 * E + n0 + nw],
                            start=(k == 0), stop=(k == KC - 1))
                    nc.vector.tensor_add(
                        vt[:, n0 // D:(n0 + nw) // D, 0:D],
                        acc[:, 0:nw].rearrange("p (h d) -> p h d", d=D),
                        biasv_bc[:, n0:n0 + nw].rearrange("p (h d) -> p h d", d=D))

            # ---------- C: head pairs
            aoT = []
            for t in range(PAIRS):
                at = work.tile([128, S], F32R, tag=f"ao{t}", name=f"aoT{t}_{b}")
                aoT.append(at)
            for t in range(PAIRS):
                # C1: q^T, k^T feature tiles for this pair
                q_r = work.tile([128, S], F32R, tag="qt", bufs=2, name=f"q{t}_{b}")
                k_r = work.tile([128, S], F32R, tag="kt", bufs=2, name=f"k{t}_{b}")
                for dst, m in ((q_r, t), (k_r, KC + t)):
                    for c0 in (0, 512):
                        acc = ps.tile([128, 512], F32, tag="acc",
                                      name=f"qk{b}_{m}_{c0}")
                        for k in range(KC):
                            nc.tensor.matmul(
                                acc, wa_r[k][:, m * 128:(m + 1) * 128],
                                xT[k][:, c0:c0 + 512],
                                start=(k == 0), stop=(k == KC - 1))
                        nc.scalar.activation(
                            dst[:, c0:c0 + 512], acc, AF.Identity,
                            bias=ba_qk[:, m:m + 1])

                # C2: the two heads of this pair
                for hh in range(2):
                    h = 2 * t + hh
                    po = hh * 64
                    av0 = ps.tile([65, 512], F32, tag="av", name=f"av0_{b}_{h}")
                    av1 = ps.tile([65, 512], F32, tag="av", name=f"av1_{b}_{h}")
                    av = (av0, av1)
                    for kb in range(ST):
                        k0 = kb * 128
                        ncols = S - k0
                        exp_t = work.tile([128, ncols], BF16, tag="exp", bufs=4,
                                          name=f"exp{b}_{h}_{kb}")
                        w = S - k0
                        if w % 512 == 128 and w > 128:
                            chunks = [(k0 + o, cw) for o, cw in
                                      zip((0, w - 640, w - 256), (512,) * ((w - 640) // 512) + (384, 256))]
                            chunks = []
                            o = k0
                            rem = w
                            while rem > 640:
                                chunks.append((o, 512)); o += 512; rem -= 512
                            chunks += [(o, 384), (o + 384, 256)]
                        else:
                            chunks = []
                            o = k0
                            while o < S:
                                cw = min(512, S - o)
                                chunks.append((o, cw)); o += cw
                        for c0, cw in chunks:
                            sc = ps.tile([128, 512], F32, tag="sc",
                                         name=f"sc{b}_{h}_{kb}_{c0}")
                            nc.tensor.matmul(
                                sc[:, 0:cw],
                                k_r[po:po + 64, k0:k0 + 128],
                                q_r[po:po + 64, c0:c0 + cw],
                                start=True, stop=True)
                            nc.scalar.activation(
                                exp_t[:, c0 - k0:c0 - k0 + cw], sc[:, 0:cw],
                                AF.Exp, scale=0.125)
                        # causal mask on the diagonal block
                        nc.vector.tensor_mul(
                            exp_t[:, 0:128], exp_t[:, 0:128], trimask)
                        # attn@v contributions of this band
                        for ci, s0 in enumerate((0, 512)):
                            if k0 < s0 + 512:
                                lo = max(s0, k0)
                                last_kb = 3 if ci == 0 else 7
                                nc.tensor.matmul(
                                    av[ci][:, lo - s0:512],
                                    v_r[kb][:, h, :],
                                    exp_t[:, lo - k0:s0 + 512 - k0],
                                    start=(kb == 0), stop=(kb == last_kb))
                    # copy denominators + unnormalized attn_out^T out of PSUM
                    # (frees the av bank before the slow reciprocal runs)
                    for ci, s0 in enumerate((0, 512)):
                        srow = work.tile([1, 512], F32, tag="srow", bufs=2,
                                         name=f"srow_{b}_{h}_{ci}")
                        nc.scalar.activation(srow, av[ci][64:65, :], AF.Copy)
                        dst = aoT[t][po:po + 64, s0:s0 + 512]
                        nc.vector.tensor_copy(dst, av[ci][0:64, :])
                        rrow = work.tile([1, 512], F32R, tag="rrow", bufs=2,
                                         name=f"rrow_{b}_{h}_{ci}")
                        with nc.allow_low_precision(reason="denom f32r"):
                            nc.vector.reciprocal(rrow, srow)
                        bc = ps.tile([128, 512], F32, tag="tr",
                                     name=f"bc{b}_{h}_{ci}")
                        nc.tensor.matmul(bc, ones_row, rrow, start=True, stop=True)
                        nc.vector.tensor_mul(dst, dst, bc[po:po + 64, :])

            # ---------- D: proj
            for st in range(ST):
                outt = work.tile([128, E], F32, tag="outt", bufs=2,
                                 name=f"outt{b}_{st}")
                for n0, nw in ((0, 512), (512, 256)):
                    acc = ps.tile([128, 512], F32, tag="acc",
                                  name=f"pacc{b}_{st}_{n0}")
                    for k in range(KC):
                        nc.tensor.matmul(
                            acc[:, 0:nw],
                            aoT[k][:, st * 128:(st + 1) * 128],
                            wp_r[k][:, n0:n0 + nw],
                            start=(k == 0), stop=(k == KC - 1))
                    nc.vector.tensor_add(outt[:, n0:n0 + nw], acc[:, 0:nw],
                                         biasp_bc[:, n0:n0 + nw])
                nc.sync.dma_start(out[b, st * 128:(st + 1) * 128, :], outt)


def build():
    from concourse import bacc

    nc = bacc.Bacc("TRN2", target_bir_lowering=False, debug=False)
    ins = {
        "hidden_states": nc.dram_tensor(
            "hidden_states", [B_LOC, S, E], F32, kind="ExternalInput").ap(),
        "W_attn": nc.dram_tensor("W_attn", [E, 3 * E], F32R,
                                 kind="ExternalInput").ap(),
        "b_attn": nc.dram_tensor("b_attn", [3 * E], F32R,
                                 kind="ExternalInput").ap(),
        "W_proj": nc.dram_tensor("W_proj", [E, E], F32R,
                                 kind="ExternalInput").ap(),
        "b_proj": nc.dram_tensor("b_proj", [E], F32R, kind="ExternalInput").ap(),
    }
    outs = {
        "out": nc.dram_tensor("out", [B_LOC, S, E], F32,
                              kind="ExternalOutput").ap(),
    }
    with tile.TileContext(nc) as tc:
        emit(tc, outs, ins)
    nc.compile()
    return nc


_CACHED_NC = None


def kernel(hidden_states, W_attn, b_attn, W_proj, b_proj, trace=False):
    global _CACHED_NC
    from concourse.bass_utils import run_bass_kernel_spmd

    if _CACHED_NC is None:
        _CACHED_NC = build()
    nc = _CACHED_NC

    hidden_states = np.ascontiguousarray(hidden_states, dtype=np.float32)
    W_attn = np.ascontiguousarray(W_attn, dtype=np.float32)
    b_attn = np.ascontiguousarray(b_attn, dtype=np.float32)
    W_proj = np.ascontiguousarray(W_proj, dtype=np.float32)
    b_proj = np.ascontiguousarray(b_proj, dtype=np.float32)

    in_maps = []
    for c in range(NCORES):
        in_maps.append({
            "hidden_states": hidden_states[c * B_LOC:(c + 1) * B_LOC],
            "W_attn": W_attn, "b_attn": b_attn,
            "W_proj": W_proj, "b_proj": b_proj,
        })
    res = run_bass_kernel_spmd(nc, in_maps, core_ids=list(range(NCORES)),
                               trace=trace)
    out = np.concatenate([res.results[c]["out"] for c in range(NCORES)], axis=0)
    kernel.last_result = res
    return out



# revision 2
# speedup vs baseline: 1.3716x; 1.3716x over previous
"""GPT-2 attention (B=16, S=1024, E=768, H=12, D=64) on 8 TRN2 NeuronCores.

Sharding: data-parallel over batch — each core processes B_LOC=2 batch
elements with fully replicated weights. No collectives.

Per-core pipeline (per batch element):
  A. x [1024,768] -> x^T in SBUF via PE transposes (48 [128,128] tiles),
     PSUM->SBUF evacuation on DVE
  B. v = x @ W_v (seq-major, 12 heads x [64 cols + ones col], NO bias —
     b_v is folded into b_proj on the host since softmax rows sum to 1)
  C. per head-pair t:
     C1. q^T, k^T = (x @ W)^T feature-major; bias added via DVE
         tensor_scalar on the PSUM->SBUF copy
     C2. scores^T bands for both heads CONCURRENTLY (row-groups 0-1 vs
         2-3 of the PE array, different PSUM banks) -> Exp(s/8) on ACT ->
         causal mask of diagonal block on GPSIMD -> attn@v in SEQ-MAJOR
         layout (lhsT=exp^T slice, rhs=v with fused ones column) so the
         softmax denominator lands as a per-partition column -> [128,1]
         reciprocal + tensor_scalar normalize on DVE -> PE transpose back
         to feature-major aoT
  D. out = attn_out @ W_proj + b_proj_eff (bias broadcast pre-built)

All matmuls run in float32r (full-rate PE).
"""

import sys

sys.path.insert(0, "/opt/trn_rl_repo")

from contextlib import ExitStack

import numpy as np

import concourse.bass as bass
import concourse.mybir as mybir
import concourse.tile as tile
from concourse.masks import make_identity

F32 = mybir.dt.float32
F32R = mybir.dt.float32r
BF16 = mybir.dt.bfloat16
AF = mybir.ActivationFunctionType
ALU = mybir.AluOpType

B, S, E = 16, 1024, 768
H, D = 12, 64
NCORES = 8
B_LOC = B // NCORES          # 2 batch elements per core
KC = E // 128                # 6 contraction chunks
ST = S // 128                # 8 seq tiles
PAIRS = H // 2               # 6 head pairs


def emit(tc, outs, ins):
    nc = tc.nc
    x, wa, ba, wp, bp = (ins["hidden_states"], ins["W_attn"], ins["b_attn"],
                         ins["W_proj"], ins["b_proj"])
    out = outs["out"]
    wa = wa if wa.dtype == F32R else wa.bitcast(F32R)
    ba = ba if ba.dtype == F32R else ba.bitcast(F32R)
    wp = wp if wp.dtype == F32R else wp.bitcast(F32R)
    bp = bp if bp.dtype == F32R else bp.bitcast(F32R)

    ctx = ExitStack()
    with ctx:
        wpool = ctx.enter_context(tc.tile_pool(name="wpool", bufs=1))
        work = ctx.enter_context(tc.tile_pool(name="work", bufs=1))
        ps = ctx.enter_context(tc.tile_pool(name="ps", bufs=2, space="PSUM"))

        # ---------- persistent weights
        wa_r = []
        for k in range(KC):
            w = wpool.tile([128, 3 * E], F32R, tag=f"wa{k}", name=f"wa{k}")
            nc.sync.dma_start(w, wa[k * 128:(k + 1) * 128, :])
            wa_r.append(w)
        wp_r = []
        for k in range(KC):
            w = wpool.tile([128, E], F32R, tag=f"wp{k}", name=f"wp{k}")
            nc.sync.dma_start(w, wp[k * 128:(k + 1) * 128, :])
            wp_r.append(w)

        # q/k bias, feature-major [128, 12]: (p, m) = b_attn[m*128 + p]
        ba_qk = wpool.tile([128, 2 * KC], F32)
        nc.sync.dma_start(ba_qk.bitcast(F32R),
                          ba[0:2 * E].rearrange("(m p) -> p m", p=128))
        # effective proj bias (b_v @ W_proj + b_proj, precomputed on host)
        bp_r = wpool.tile([1, E], F32R)
        nc.sync.dma_start(bp_r, bp.unsqueeze(0))

        identity = wpool.tile([128, 128], F32)
        make_identity(nc, identity)

        ones_row32 = wpool.tile([1, 128], F32)
        nc.vector.memset(ones_row32, 1.0)
        ones_row = wpool.tile([1, 128], F32R)
        nc.vector.tensor_copy(ones_row, ones_row32)

        # broadcast proj bias to [128, E] so the bias-add is a plain TT add
        biasp_bc = wpool.tile([128, E], F32)
        for n0, nw in ((0, 512), (512, 256)):
            bps = ps.tile([128, 512], F32, tag="acc", name=f"bbc{n0}")
            nc.tensor.matmul(bps[:, 0:nw], ones_row, bp_r[0:1, n0:n0 + nw],
                             start=True, stop=True)
            nc.scalar.activation(biasp_bc[:, n0:n0 + nw], bps[:, 0:nw], AF.Copy)

        # causal tri-mask [128,128]: mask[r,c] = 1 if c >= r else 0
        trimask32 = wpool.tile([128, 128], F32)
        nc.gpsimd.memset(trimask32, 1.0)
        nc.gpsimd.affine_select(
            out=trimask32, in_=trimask32, compare_op=ALU.is_ge,
            fill=0.0, base=0, pattern=[[1, 128]], channel_multiplier=-1,
        )
        trimask = wpool.tile([128, 128], BF16)
        nc.vector.tensor_copy(trimask, trimask32)

        for b in range(B_LOC):
            # ---------- A: x^T via PE transposes
            xT = []
            for k in range(KC):
                t_ = work.tile([128, S], F32R, tag=f"xt{k}", name=f"xT{k}_{b}")
                xT.append(t_)
            for k in range(KC):
                for st in range(ST):
                    xin = work.tile([128, 128], F32, tag="xin", bufs=3,
                                    name=f"xin{b}_{k}_{st}")
                    nc.sync.dma_start(
                        xin, x[b, st * 128:(st + 1) * 128, k * 128:(k + 1) * 128])
                    tr_ps = ps.tile([128, 512], F32, tag="acc",
                                    name=f"tr{b}_{k}_{st}")
                    nc.tensor.transpose(tr_ps[:, 0:128], xin, identity)
                    nc.vector.tensor_copy(
                        xT[k][:, st * 128:(st + 1) * 128], tr_ps[:, 0:128])

            # ---------- B: v (seq-major, 12 heads x [64 cols + ones col])
            v_r = []
            for st in range(ST):
                vt = work.tile([128, H, D + 1], BF16, tag=f"v{st}", name=f"v{st}_{b}")
                v_r.append(vt)
                nc.gpsimd.memset(vt[:, :, D:D + 1], 1.0)
                for n0, nw in ((0, 512), (512, 256)):
                    acc = ps.tile([128, 512], F32, tag="acc",
                                  name=f"vacc{b}_{st}_{n0}")
                    for k in range(KC):
                        nc.tensor.matmul(
                            acc[:, 0:nw],
                            xT[k][:, st * 128:(st + 1) * 128],
                            wa_r[k][:, 2 * E + n0:2 * E + n0 + nw],
                            start=(k == 0), stop=(k == KC - 1))
                    nc.vector.tensor_copy(
                        vt[:, n0 // D:(n0 + nw) // D, 0:D],
                        acc[:, 0:nw].rearrange("p (h d) -> p h d", d=D))

            # ---------- C: head pairs
            aoT = []
            for t in range(PAIRS):
                at = work.tile([128, S], F32R, tag=f"ao{t}", name=f"aoT{t}_{b}")
                aoT.append(at)
            for t in range(PAIRS):
                # C1: q^T, k^T feature tiles for this pair
                q_r = work.tile([128, S], F32R, tag="qt", bufs=2, name=f"q{t}_{b}")
                k_r = work.tile([128, S], F32R, tag="kt", bufs=2, name=f"k{t}_{b}")
                for dst, m in ((q_r, t), (k_r, KC + t)):
                    for c0 in (0, 512):
                        acc = ps.tile([128, 512], F32, tag="acc",
                                      name=f"qk{b}_{m}_{c0}")
                        for k in range(KC):
                            nc.tensor.matmul(
                                acc, wa_r[k][:, m * 128:(m + 1) * 128],
                                xT[k][:, c0:c0 + 512],
                                start=(k == 0), stop=(k == KC - 1))
                        nc.vector.tensor_scalar_add(
                            dst[:, c0:c0 + 512], acc, ba_qk[:, m:m + 1])

                # C2a: scores + exp, both heads interleaved (concurrent
                # row-group matmuls: head 0 uses partitions 0-63, head 1
                # uses 64-127, alternating PSUM banks via the sc ring)
                exp_t = [[None] * ST for _ in range(2)]
                for hh in range(2):
                    for kb in range(ST):
                        k0 = kb * 128
                        exp_t[hh][kb] = work.tile(
                            [128, S - k0], BF16, tag=f"exp{hh}_{kb}", bufs=2,
                            name=f"exp{b}_{t}_{hh}_{kb}")
                for kb in range(ST):
                    k0 = kb * 128
                    w = S - k0
                    chunks = [(0, min(512, w))]
                    if w > 512:
                        chunks.append((512, w - 512))
                    for off, cw in chunks:
                        for hh in range(2):
                            po = hh * 64
                            sc = ps.tile([128, 512], F32, tag="sc",
                                         name=f"sc{b}_{t}_{hh}_{kb}_{off}")
                            nc.tensor.matmul(
                                sc[:, 0:cw],
                                k_r[po:po + 64, k0:k0 + 128],
                                q_r[po:po + 64, k0 + off:k0 + off + cw],
                                start=True, stop=True)
                            nc.scalar.activation(
                                exp_t[hh][kb][:, off:off + cw], sc[:, 0:cw],
                                AF.Exp, scale=0.125)
                    # causal mask on the diagonal block (GPSIMD, SBUF only)
                    for hh in range(2):
                        nc.gpsimd.tensor_mul(
                            exp_t[hh][kb][:, 0:128], exp_t[hh][kb][:, 0:128],
                            trimask)

                # C2b: attn@v seq-major; heads in separate PSUM banks
                for qt in range(ST):
                    av = ps.tile([128, 1024], F32, tag="av",
                                 name=f"av{b}_{t}_{qt}")
                    for hh in range(2):
                        h = 2 * t + hh
                        co = hh * 512
                        for kb in range(qt + 1):
                            nc.tensor.matmul(
                                av[:, co:co + 65],
                                exp_t[hh][kb][:, (qt - kb) * 128:(qt - kb + 1) * 128],
                                v_r[kb][:, h, :],
                                start=(kb == 0), stop=(kb == qt))
                    rec = work.tile([128, 2], F32, tag="rec", bufs=2,
                                    name=f"rec{b}_{t}_{qt}")
                    ao_p = work.tile([128, 128], F32, tag="aop", bufs=2,
                                     name=f"aop{b}_{t}_{qt}")
                    for hh in range(2):
                        co = hh * 512
                        nc.vector.reciprocal(
                            rec[:, hh:hh + 1], av[:, co + 64:co + 65])
                        nc.vector.tensor_scalar_mul(
                            ao_p[:, hh * 64:(hh + 1) * 64],
                            av[:, co:co + 64], rec[:, hh:hh + 1])
                    # transpose back to feature-major aoT
                    tr2 = ps.tile([128, 1024], F32, tag="av",
                                  name=f"aotr{b}_{t}_{qt}")
                    nc.tensor.transpose(tr2[:, 0:128], ao_p, identity)
                    nc.vector.tensor_copy(
                        aoT[t][:, qt * 128:(qt + 1) * 128], tr2[:, 0:128])

            # ---------- D: proj
            for st in range(ST):
                outt = work.tile([128, E], F32, tag="outt", bufs=2,
                                 name=f"outt{b}_{st}")
                for n0, nw in ((0, 512), (512, 256)):
                    acc = ps.tile([128, 512], F32, tag="acc",
                                  name=f"pacc{b}_{st}_{n0}")
                    for k in range(KC):
                        nc.tensor.matmul(
                            acc[:, 0:nw],
                            aoT[k][:, st * 128:(st + 1) * 128],
                            wp_r[k][:, n0:n0 + nw],
                            start=(k == 0), stop=(k == KC - 1))
                    nc.vector.tensor_add(outt[:, n0:n0 + nw], acc[:, 0:nw],
                                         biasp_bc[:, n0:n0 + nw])
                nc.sync.dma_start(out[b, st * 128:(st + 1) * 128, :], outt)


def build():
    from concourse import bacc

    nc = bacc.Bacc("TRN2", target_bir_lowering=False, debug=False)
    ins = {
        "hidden_states": nc.dram_tensor(
            "hidden_states", [B_LOC, S, E], F32, kind="ExternalInput").ap(),
        "W_attn": nc.dram_tensor("W_attn", [E, 3 * E], F32R,
                                 kind="ExternalInput").ap(),
        "b_attn": nc.dram_tensor("b_attn", [3 * E], F32R,
                                 kind="ExternalInput").ap(),
        "W_proj": nc.dram_tensor("W_proj", [E, E], F32R,
                                 kind="ExternalInput").ap(),
        "b_proj": nc.dram_tensor("b_proj", [E], F32R, kind="ExternalInput").ap(),
    }
    outs = {
        "out": nc.dram_tensor("out", [B_LOC, S, E], F32,
                              kind="ExternalOutput").ap(),
    }
    with tile.TileContext(nc) as tc:
        emit(tc, outs, ins)
    nc.compile()
    return nc


_CACHED_NC = None


def kernel(hidden_states, W_attn, b_attn, W_proj, b_proj, trace=False):
    global _CACHED_NC
    from concourse.bass_utils import run_bass_kernel_spmd

    if _CACHED_NC is None:
        _CACHED_NC = build()
    nc = _CACHED_NC

    hidden_states = np.ascontiguousarray(hidden_states, dtype=np.float32)
    W_attn = np.ascontiguousarray(W_attn, dtype=np.float32)
    b_attn = np.ascontiguousarray(b_attn, dtype=np.float32)
    W_proj = np.ascontiguousarray(W_proj, dtype=np.float32)
    b_proj = np.ascontiguousarray(b_proj, dtype=np.float32)

    # fold the v-bias into the proj bias: softmax rows sum to 1, so
    # attn @ (v + b_v) @ Wp + b_p == attn @ v @ Wp + (b_v @ Wp + b_p)
    bp_eff = np.ascontiguousarray(
        b_attn[2 * E:3 * E] @ W_proj + b_proj, dtype=np.float32)

    in_maps = []
    for c in range(NCORES):
        in_maps.append({
            "hidden_states": hidden_states[c * B_LOC:(c + 1) * B_LOC],
            "W_attn": W_attn, "b_attn": b_attn,
            "W_proj": W_proj, "b_proj": bp_eff,
        })
    res = run_bass_kernel_spmd(nc, in_maps, core_ids=list(range(NCORES)),
                               trace=trace)
    out = np.concatenate([res.results[c]["out"] for c in range(NCORES)], axis=0)
    kernel.last_result = res
    return out
